# revision 20
# baseline (speedup 1.0000x reference)
"""EntropyDispatchedLinear (int8-weight GEMM with per-column dequant) on 8 TRN2 cores.

out[m, n] = (sum_k x[m, k] * w_int8[k, n]) * w_scale[n],  x fp16 [32, 8192],
w_int8 int8 [8192, 28672], out fp16 [32, 28672].

Strategy (tensor-parallel over out_features N, 3584 columns per core), measured
at 77.1 us/rep steady-state vs 99.6 us for the prior baseline:
- Weight shard streamed HBM->SBUF as TWO disjoint int8 strip streams from a
  host-repacked contiguous layout, both on the SP HWDGE ring: a DVE stream
  (cols [0,2048), 8192B/partition/strip) and an ACT stream (cols [2048,3584),
  6144B/partition/strip). Single contiguous per-partition descriptors measure
  366+ GB/s vs 245 GB/s for the strided descriptors of the naive [K, NS]
  layout; 16 strips x ~1MB per stream keeps per-DMA overhead hidden.
- On-chip upconvert int8 -> bf16 (exact) with FULLY DISJOINT engine pipelines:
  DVE tensor_copy (2.10 el/ns/part) converts its stream, ACT copy (1.376)
  converts its own; separate src AND dst tiles per engine. Any tile shared
  between the two engines (or a dma_start issued from the ACT sequencer, which
  FIFO-blocks its own copies) serializes the pipeline and costs 20-50 us.
- Matmuls: stationary = x^T k-tile [128, 32] fp16 (host-transposed, replicated),
  moving = converted bf16 weight tile [128, 512]. M=32 only fills 32 PE columns,
  so 7 n-tiles are packed into 2 PSUM banks at column offsets 0/32/64/96
  (tile_position col packing; concurrent col-group matmuls measured 90 ns/MM
  effective = 40 us for the full 448-MM schedule). PSUM and out tiles are
  double-buffered so the next rep's matmuls overlap the epilogue.
- Epilogue: psum * scale (DVE tensor_mul, scale pre-broadcast host-side to the
  packed psum layout), fp16 out, strided DMA per psum bank on gpsimd (SWDGE),
  which is off the critical path and keeps both HWDGE rings clear.
"""
import numpy as np

M, K, NFULL = 32, 8192, 28672
NCORES = 8
NS = NFULL // NCORES          # 3584 columns per core
KT = K // 128                 # 64 k-tiles
STRIP_KT = 8                  # k-tiles per DMA strip
NSTRIP = KT // STRIP_KT       # 16
DVE_END = 2048                # DVE converts [0, DVE_END), ACT [DVE_END, NS)
NT = NS // 512                # 7 n-tiles
NTD = DVE_END // 512          # n-tiles fed from the DVE-converted tile

_CACHE = {}


def _build(reps=1):
    import concourse.bacc as bacc
    import concourse.mybir as mybir
    import concourse.tile as tile

    nc = bacc.Bacc("TRN2", target_bir_lowering=False, debug=False, num_devices=NCORES)
    dt = mybir.dt
    xq = nc.dram_tensor("xq", [128, KT * M], dt.float16, kind="ExternalInput").ap()
    w8d = nc.dram_tensor("w8d", [NSTRIP, 128, STRIP_KT * DVE_END], dt.int8, kind="ExternalInput").ap()
    w8a = nc.dram_tensor("w8a", [NSTRIP, 128, STRIP_KT * (NS - DVE_END)], dt.int8, kind="ExternalInput").ap()
    scaleA = nc.dram_tensor("scaleA", [128, 512], dt.float32, kind="ExternalInput").ap()
    scaleB = nc.dram_tensor("scaleB", [128, 512], dt.float32, kind="ExternalInput").ap()
    out = nc.dram_tensor("out", [M, NS], dt.float16, kind="ExternalOutput").ap()

    with tile.TileContext(nc) as tc:
        with (
            tc.tile_pool(name="xp", bufs=1) as xp,
            tc.tile_pool(name="sp", bufs=1) as scp,
            tc.tile_pool(name="wraw", bufs=3) as wrawp,
            tc.tile_pool(name="wbf", bufs=2) as wbfp,
            tc.tile_pool(name="op", bufs=1) as outp,
            tc.tile_pool(name="ps", bufs=2, space="PSUM") as psp,
        ):
            # preloads on gpsimd so the sync HWDGE ring starts weight strips
            # immediately on a single-shot run
            xsb = xp.tile([128, KT, M], dt.float16, tag="x")
            nc.gpsimd.dma_start(xsb.rearrange("p a b -> p (a b)"), xq)
            scA = scp.tile([128, 512], dt.float32, tag="scA")
            nc.gpsimd.dma_start(scA[:], scaleA)
            scB = scp.tile([128, 512], dt.float32, tag="scB")
            nc.gpsimd.dma_start(scB[:], scaleB)

            def body():
                pA = psp.tile([128, 512], dt.float32, tag="pA")
                pB = psp.tile([128, 512], dt.float32, tag="pB")
                for s in range(NSTRIP):
                    # fully disjoint DVE and ACT conversion pipelines: separate
                    # HBM streams, separate HWDGE rings, separate src/dst tiles
                    # (shared tiles between the two engines measurably interfere)
                    wrawD = wrawp.tile([128, STRIP_KT, DVE_END], dt.int8, tag="wrawD")
                    nc.sync.dma_start(wrawD.rearrange("p a b -> p (a b)"), w8d[s])
                    wrawA = wrawp.tile([128, STRIP_KT, NS - DVE_END], dt.int8, tag="wrawA")
                    nc.sync.dma_start(wrawA.rearrange("p a b -> p (a b)"), w8a[s])
                    wbfD = wbfp.tile([128, STRIP_KT, DVE_END], dt.bfloat16, tag="wbfD")
                    nc.vector.tensor_copy(wbfD[:], wrawD[:])
                    wbfA = wbfp.tile([128, STRIP_KT, NS - DVE_END], dt.bfloat16, tag="wbfA")
                    nc.scalar.copy(wbfA[:], wrawA[:])
                    for t in range(STRIP_KT):
                        kt = s * STRIP_KT + t
                        for nt in range(NT):
                            p, j = (pA, nt) if nt < 4 else (pB, nt - 4)
                            if nt < NTD:
                                mv = wbfD[:, t, 512 * nt:512 * (nt + 1)]
                            else:
                                mv = wbfA[:, t, 512 * (nt - NTD):512 * (nt - NTD + 1)]
                            nc.tensor.matmul(
                                p[32 * j:32 * j + 32, :],
                                xsb[:, kt, :],
                                mv,
                                start=(kt == 0),
                                stop=(kt == KT - 1),
                                tile_position=(0, 32 * j),
                            )
                oA = outp.tile([128, 512], dt.float16, tag="oA")
                nc.vector.tensor_mul(oA[:], pA[:], scA[:])
                oB = outp.tile([96, 512], dt.float16, tag="oB")
                nc.vector.tensor_mul(oB[:], pB[0:96, :], scB[0:96, :])
                outA_view = out[:, 0:2048].rearrange("m (j f) -> j m f", f=512)
                nc.gpsimd.dma_start(outA_view, oA[:])
                outB_view = out[:, 2048:NS].rearrange("m (j f) -> j m f", f=512)
                nc.gpsimd.dma_start(outB_view, oB[:])

            if reps == 1:
                body()
            else:
                with tc.For_i(0, reps, 1):
                    body()
    nc.compile()
    return nc


def get_nc(reps=1):
    if reps not in _CACHE:
        _CACHE[reps] = _build(reps)
    return _CACHE[reps]


def shard_inputs(x, w_int8, w_scale):
    """Full inputs -> list of 8 per-core input dicts (host-side shard/repack)."""
    x = np.asarray(x)
    if x.dtype != np.float16:
        x = x.astype(np.float16)
    w_int8 = np.asarray(w_int8)
    if w_int8.dtype != np.int8:
        w_int8 = w_int8.astype(np.int8)
    w_scale = np.asarray(w_scale)
    if w_scale.dtype != np.float32:
        w_scale = w_scale.astype(np.float32)
    x2d = x.reshape(-1, K)
    assert x2d.shape == (M, K), f"unexpected x shape {x.shape}"
    # xq[p, kt*M + m] = x[m, kt*128 + p]
    xq = np.ascontiguousarray(
        x2d.T.reshape(KT, 128, M).transpose(1, 0, 2).reshape(128, KT * M))
    in_maps = []
    for c in range(NCORES):
        ws = w_scale[c * NS:(c + 1) * NS]
        scA = np.empty((128, 512), np.float32)
        scB = np.zeros((128, 512), np.float32)
        for j in range(4):
            scA[32 * j:32 * j + 32, :] = ws[512 * j:512 * (j + 1)][None, :]
        for j in range(3):
            scB[32 * j:32 * j + 32, :] = ws[2048 + 512 * j:2048 + 512 * (j + 1)][None, :]
        # w8d[s, p, t*DVE_END + n] = w_int8[(s*STRIP_KT + t)*128 + p, c*NS + n]
        # (DVE columns), w8a likewise for the ACT columns [DVE_END, NS)
        wc = w_int8[:, c * NS:(c + 1) * NS].reshape(NSTRIP, STRIP_KT, 128, NS)
        w8dc = np.ascontiguousarray(
            wc[:, :, :, 0:DVE_END].transpose(0, 2, 1, 3)
            .reshape(NSTRIP, 128, STRIP_KT * DVE_END))
        w8ac = np.ascontiguousarray(
            wc[:, :, :, DVE_END:NS].transpose(0, 2, 1, 3)
            .reshape(NSTRIP, 128, STRIP_KT * (NS - DVE_END)))
        in_maps.append({
            "xq": xq,
            "w8d": w8dc,
            "w8a": w8ac,
            "scaleA": scA,
            "scaleB": scB,
        })
    return in_maps


def kernel(x, w_int8, w_scale):
    """Full unsharded inputs -> full [32, 28672] fp16 output (8-core TRN2)."""
    from concourse.bass_utils import run_bass_kernel_spmd

    orig_shape = np.asarray(x).shape[:-1] + (NFULL,)
    nc = get_nc(reps=1)
    in_maps = shard_inputs(x, w_int8, w_scale)
    res = run_bass_kernel_spmd(nc, in_maps, core_ids=list(range(NCORES))).results
    out = np.concatenate([res[c]["out"] for c in range(NCORES)], axis=1)
    return out.reshape(orig_shape)


# revision 22
# speedup vs baseline: 1.0541x; 1.0541x over previous
"""EntropyDispatchedLinear (int8-weight GEMM with per-column dequant) on 8 TRN2 cores.

out[m, n] = (sum_k x[m, k] * w_int8[k, n]) * w_scale[n],  x fp16 [32, 8192],
w_int8 int8 [8192, 28672], out fp16 [32, 28672].

Strategy (tensor-parallel over out_features N, 3584 columns per core), measured
at 77.1 us/rep steady-state vs 99.6 us for the prior baseline:
- Weight shard streamed HBM->SBUF as TWO disjoint int8 strip streams from a
  host-repacked contiguous layout, both on the SP HWDGE ring: a DVE stream
  (cols [0,2048), 8192B/partition/strip) and an ACT stream (cols [2048,3584),
  6144B/partition/strip). Single contiguous per-partition descriptors measure
  366+ GB/s vs 245 GB/s for the strided descriptors of the naive [K, NS]
  layout; 16 strips x ~1MB per stream keeps per-DMA overhead hidden.
- On-chip upconvert int8 -> bf16 (exact) with FULLY DISJOINT engine pipelines:
  DVE tensor_copy (2.10 el/ns/part) converts its stream, ACT copy (1.376)
  converts its own; separate src AND dst tiles per engine. Any tile shared
  between the two engines (or a dma_start issued from the ACT sequencer, which
  FIFO-blocks its own copies) serializes the pipeline and costs 20-50 us.
- Matmuls: stationary = x^T k-tile [128, 32] fp16 (host-transposed, replicated),
  moving = converted bf16 weight tile [128, 512]. M=32 only fills 32 PE columns,
  so 7 n-tiles are packed into 2 PSUM banks at column offsets 0/32/64/96
  (tile_position col packing; concurrent col-group matmuls measured 90 ns/MM
  effective = 40 us for the full 448-MM schedule). PSUM and out tiles are
  double-buffered so the next rep's matmuls overlap the epilogue.
- Epilogue: psum * scale (DVE tensor_mul, scale pre-broadcast host-side to the
  packed psum layout), fp16 out, strided DMA per psum bank on gpsimd (SWDGE),
  which is off the critical path and keeps both HWDGE rings clear.
"""
import numpy as np

M, K, NFULL = 32, 8192, 28672
NCORES = 8
NS = NFULL // NCORES          # 3584 columns per core
KT = K // 128                 # 64 k-tiles
STRIP_KT = 4                  # k-tiles per DMA strip
NSTRIP = KT // STRIP_KT       # 16
DVE_END = 2048                # DVE converts [0, DVE_END), ACT [DVE_END, NS)
NT = NS // 512                # 7 n-tiles
NTD = DVE_END // 512          # n-tiles fed from the DVE-converted tile

_CACHE = {}


def _build(reps=1):
    import concourse.bacc as bacc
    import concourse.mybir as mybir
    import concourse.tile as tile

    nc = bacc.Bacc("TRN2", target_bir_lowering=False, debug=False, num_devices=NCORES)
    dt = mybir.dt
    xq = nc.dram_tensor("xq", [128, KT * M], dt.float16, kind="ExternalInput").ap()
    w8d = nc.dram_tensor("w8d", [NSTRIP, 128, STRIP_KT * DVE_END], dt.int8, kind="ExternalInput").ap()
    w8a = nc.dram_tensor("w8a", [NSTRIP, 128, STRIP_KT * (NS - DVE_END)], dt.int8, kind="ExternalInput").ap()
    scaleA = nc.dram_tensor("scaleA", [128, 512], dt.float32, kind="ExternalInput").ap()
    scaleB = nc.dram_tensor("scaleB", [128, 512], dt.float32, kind="ExternalInput").ap()
    out = nc.dram_tensor("out", [M, NS], dt.float16, kind="ExternalOutput").ap()

    with tile.TileContext(nc) as tc:
        with (
            tc.tile_pool(name="xp", bufs=1) as xp,
            tc.tile_pool(name="sp", bufs=1) as scp,
            tc.tile_pool(name="wraw", bufs=6) as wrawp,
            tc.tile_pool(name="wbf", bufs=3) as wbfp,
            tc.tile_pool(name="op", bufs=2) as outp,
            tc.tile_pool(name="ps", bufs=2, space="PSUM") as psp,
        ):
            # preloads on gpsimd so the sync HWDGE ring starts weight strips
            # immediately on a single-shot run
            xsb = xp.tile([128, KT, M], dt.float16, tag="x")
            nc.gpsimd.dma_start(xsb.rearrange("p a b -> p (a b)"), xq)
            scA = scp.tile([128, 512], dt.float32, tag="scA")
            nc.gpsimd.dma_start(scA[:], scaleA)
            scB = scp.tile([128, 512], dt.float32, tag="scB")
            nc.gpsimd.dma_start(scB[:], scaleB)

            def body():
                pA = psp.tile([128, 512], dt.float32, tag="pA")
                pB = psp.tile([128, 512], dt.float32, tag="pB")
                for s in range(NSTRIP):
                    # fully disjoint DVE and ACT conversion pipelines: separate
                    # HBM streams, separate HWDGE rings, separate src/dst tiles
                    # (shared tiles between the two engines measurably interfere)
                    wrawD = wrawp.tile([128, STRIP_KT, DVE_END], dt.int8, tag="wrawD")
                    nc.sync.dma_start(wrawD.rearrange("p a b -> p (a b)"), w8d[s])
                    wrawA = wrawp.tile([128, STRIP_KT, NS - DVE_END], dt.int8, tag="wrawA")
                    nc.sync.dma_start(wrawA.rearrange("p a b -> p (a b)"), w8a[s])
                    wbfD = wbfp.tile([128, STRIP_KT, DVE_END], dt.bfloat16, tag="wbfD")
                    nc.vector.tensor_copy(wbfD[:], wrawD[:])
                    wbfA = wbfp.tile([128, STRIP_KT, NS - DVE_END], dt.bfloat16, tag="wbfA")
                    nc.scalar.copy(wbfA[:], wrawA[:])
                    for t in range(STRIP_KT):
                        kt = s * STRIP_KT + t
                        for nt in range(NT):
                            p, j = (pA, nt) if nt < 4 else (pB, nt - 4)
                            if nt < NTD:
                                mv = wbfD[:, t, 512 * nt:512 * (nt + 1)]
                            else:
                                mv = wbfA[:, t, 512 * (nt - NTD):512 * (nt - NTD + 1)]
                            nc.tensor.matmul(
                                p[32 * j:32 * j + 32, :],
                                xsb[:, kt, :],
                                mv,
                                start=(kt == 0),
                                stop=(kt == KT - 1),
                                tile_position=(0, 32 * j),
                            )
                oA = outp.tile([128, 512], dt.float16, tag="oA")
                nc.vector.tensor_mul(oA[:], pA[:], scA[:])
                oB = outp.tile([96, 512], dt.float16, tag="oB")
                nc.vector.tensor_mul(oB[:], pB[0:96, :], scB[0:96, :])
                outA_view = out[:, 0:2048].rearrange("m (j f) -> j m f", f=512)
                nc.gpsimd.dma_start(outA_view, oA[:])
                outB_view = out[:, 2048:NS].rearrange("m (j f) -> j m f", f=512)
                nc.gpsimd.dma_start(outB_view, oB[:])

            if reps == 1:
                body()
            else:
                with tc.For_i(0, reps, 1):
                    body()
    nc.compile()
    return nc


def get_nc(reps=1):
    if reps not in _CACHE:
        _CACHE[reps] = _build(reps)
    return _CACHE[reps]


def shard_inputs(x, w_int8, w_scale):
    """Full inputs -> list of 8 per-core input dicts (host-side shard/repack)."""
    x = np.asarray(x)
    if x.dtype != np.float16:
        x = x.astype(np.float16)
    w_int8 = np.asarray(w_int8)
    if w_int8.dtype != np.int8:
        w_int8 = w_int8.astype(np.int8)
    w_scale = np.asarray(w_scale)
    if w_scale.dtype != np.float32:
        w_scale = w_scale.astype(np.float32)
    x2d = x.reshape(-1, K)
    assert x2d.shape == (M, K), f"unexpected x shape {x.shape}"
    # xq[p, kt*M + m] = x[m, kt*128 + p]
    xq = np.ascontiguousarray(
        x2d.T.reshape(KT, 128, M).transpose(1, 0, 2).reshape(128, KT * M))
    in_maps = []
    for c in range(NCORES):
        ws = w_scale[c * NS:(c + 1) * NS]
        scA = np.empty((128, 512), np.float32)
        scB = np.zeros((128, 512), np.float32)
        for j in range(4):
            scA[32 * j:32 * j + 32, :] = ws[512 * j:512 * (j + 1)][None, :]
        for j in range(3):
            scB[32 * j:32 * j + 32, :] = ws[2048 + 512 * j:2048 + 512 * (j + 1)][None, :]
        # w8d[s, p, t*DVE_END + n] = w_int8[(s*STRIP_KT + t)*128 + p, c*NS + n]
        # (DVE columns), w8a likewise for the ACT columns [DVE_END, NS)
        wc = w_int8[:, c * NS:(c + 1) * NS].reshape(NSTRIP, STRIP_KT, 128, NS)
        w8dc = np.ascontiguousarray(
            wc[:, :, :, 0:DVE_END].transpose(0, 2, 1, 3)
            .reshape(NSTRIP, 128, STRIP_KT * DVE_END))
        w8ac = np.ascontiguousarray(
            wc[:, :, :, DVE_END:NS].transpose(0, 2, 1, 3)
            .reshape(NSTRIP, 128, STRIP_KT * (NS - DVE_END)))
        in_maps.append({
            "xq": xq,
            "w8d": w8dc,
            "w8a": w8ac,
            "scaleA": scA,
            "scaleB": scB,
        })
    return in_maps


def kernel(x, w_int8, w_scale):
    """Full unsharded inputs -> full [32, 28672] fp16 output (8-core TRN2)."""
    from concourse.bass_utils import run_bass_kernel_spmd

    orig_shape = np.asarray(x).shape[:-1] + (NFULL,)
    nc = get_nc(reps=1)
    in_maps = shard_inputs(x, w_int8, w_scale)
    res = run_bass_kernel_spmd(nc, in_maps, core_ids=list(range(NCORES))).results
    out = np.concatenate([res[c]["out"] for c in range(NCORES)], axis=1)
    return out.reshape(orig_shape)
